# revision 2
# baseline (speedup 1.0000x reference)
"""Trainium2 Bass kernel for nn_A2CDense (dense GNN message-passing block).

Data-parallel over the graph-batch dim B=64: 8 graphs per NeuronCore, 8 cores.
All matmuls run in float32r (full-rate tf32-like PE mode, ~2.4e-4 product
rounding); PSUM accumulation is fp32. The dense one-hot incidence matmuls
stream E_s / E_r straight from HBM, which is the memory-bound term this
kernel is organized around.

Key algebraic restructure vs the reference: the edge-MLP first layer is
computed as

  h1 = W1_Ea @ E_a + (W1_snd @ V_a) @ E_s + (W1_rcv @ V_a) @ E_r + (b1 + W1_u @ u)

(associativity is exact because E_s / E_r are column-selectors), which avoids
materializing the gathered sender/receiver features, and folds the broadcast
global-feature term into the activation bias. The same trick is used for the
node MLP (V_a / agg / u) and the global MLP (u / sumV / sumE).
"""
import numpy as np

B, NV, NE = 64, 512, 2048
DF = 32            # all feature dims (IN_U/IN_VA/IN_EA/OUT_*/NH)
N_CORES = 8
GPC = B // N_CORES  # graphs per core
ET = 4              # edge tiles per graph (512 edges each)
ETW = NE // ET      # 512
VC = NV // 128      # 4 v-chunks

_cached = {}


def _build_nc():
    import concourse.bass as bass
    import concourse.bacc as bacc
    import concourse.tile as tile
    import concourse.mybir as mybir
    from concourse.masks import make_identity

    f32 = mybir.dt.float32
    f32r = mybir.dt.float32r
    AF = mybir.ActivationFunctionType

    nc = bacc.Bacc("TRN2", target_bir_lowering=False, debug=False,
                   num_devices=N_CORES)

    E_s = nc.dram_tensor("E_s", [GPC, NV, NE], f32r, kind="ExternalInput")
    E_r = nc.dram_tensor("E_r", [GPC, NV, NE], f32r, kind="ExternalInput")
    E_a = nc.dram_tensor("E_a", [GPC, DF, NE], f32r, kind="ExternalInput")
    V_a = nc.dram_tensor("V_a", [GPC, DF, NV], f32r, kind="ExternalInput")
    uT = nc.dram_tensor("uT", [DF, GPC], f32r, kind="ExternalInput")
    # 19 packed stationaries, each [k=32, m=32] (pre-transposed on host)
    Wk = nc.dram_tensor("Wk", [DF, 19, DF], f32r, kind="ExternalInput")
    # 12 bias vectors [32] each: fe b1-4, fv b1-4, fu b1-4
    BI = nc.dram_tensor("BI", [DF, 12], f32, kind="ExternalInput")

    out_E = nc.dram_tensor("out_E", [GPC, DF, NE], f32r, kind="ExternalOutput")
    out_V = nc.dram_tensor("out_V", [GPC, DF, NV], f32r, kind="ExternalOutput")
    out_uT = nc.dram_tensor("out_uT", [DF, GPC], f32r, kind="ExternalOutput")

    # Wk indices
    FE_EA, FE_SND, FE_RCV, FE_U, FE_W2, FE_W3, FE_W4 = 0, 1, 2, 3, 4, 5, 6
    FV_VA, FV_AGG, FV_U, FV_W2, FV_W3, FV_W4 = 7, 8, 9, 10, 11, 12
    FU_U, FU_VS, FU_ES, FU_W2, FU_W3, FU_W4 = 13, 14, 15, 16, 17, 18

    with tile.TileContext(nc) as tc:
        with (
            tc.tile_pool(name="consts", bufs=1) as consts,
            tc.tile_pool(name="work", bufs=1) as work,
        ):
            W = consts.tile([DF, 19, DF], f32r, tag="W")
            BIt = consts.tile([DF, 12], f32, tag="BI")
            u_all = consts.tile([DF, GPC], f32r, tag="u_all")
            ident = consts.tile([128, 128], f32, tag="ident")
            ident_r = consts.tile([128, 128], f32r, tag="ident_r")
            nc.sync.dma_start(W[:], Wk.ap())
            nc.sync.dma_start(BIt[:], BI.ap())
            nc.sync.dma_start(u_all[:], uT.ap())
            make_identity(nc, ident[:])
            nc.vector.tensor_copy(ident_r[:], ident[:])

            # folded L1 biases for fe / fv: b1' = b1 + W1_u @ u  -> [32, GPC] fp32
            b1e_all = work.tile([DF, GPC], f32, tag="b1e")
            b1v_all = work.tile([DF, GPC], f32, tag="b1v")
            with tc.tile_pool(name="pfold", bufs=1, space="PSUM") as pfold:
                ps_be = pfold.tile([DF, GPC], f32, tag="pbe")
                nc.tensor.matmul(ps_be[:], W[:, FE_U, :], u_all[:],
                                 start=True, stop=True)
                nc.vector.tensor_scalar_add(b1e_all[:], ps_be[:], BIt[:, 0:1])
                ps_bv = pfold.tile([DF, GPC], f32, tag="pbv")
                nc.tensor.matmul(ps_bv[:], W[:, FV_U, :], u_all[:],
                                 start=True, stop=True)
                nc.vector.tensor_scalar_add(b1v_all[:], ps_bv[:], BIt[:, 4:5])

            # per-graph sum accumulators (written column-by-column)
            oVsum = work.tile([DF, GPC], f32, tag="oVsum")
            oEsum = work.tile([DF, GPC], f32, tag="oEsum")

            with (
                tc.tile_pool(name="big", bufs=3) as big,
                tc.tile_pool(name="med", bufs=2) as med,
                tc.tile_pool(name="small", bufs=3) as small,
                tc.tile_pool(name="ph", bufs=2, space="PSUM") as php,
                tc.tile_pool(name="pagg", bufs=2, space="PSUM") as paggp,
                tc.tile_pool(name="pert", bufs=2, space="PSUM") as pertp,
                tc.tile_pool(name="pmisc", bufs=1, space="PSUM") as pmiscp,
                tc.tile_pool(name="poet", bufs=1, space="PSUM") as poetp,
            ):
                for g in range(GPC):
                    va_t = med.tile([DF, NV], f32r, tag="va")
                    ea_t = med.tile([DF, NE], f32r, tag="ea")
                    nc.sync.dma_start(va_t[:], V_a.ap()[g])
                    nc.sync.dma_start(ea_t[:], E_a.ap()[g])

                    # Msr = [W1_snd @ V_a ; W1_rcv @ V_a]  [64, 512]
                    ps_msr = pmiscp.tile([2 * DF, NV], f32, tag="pmsr")
                    nc.tensor.matmul(ps_msr[:], W[:, FE_SND:FE_RCV + 1, :],
                                     va_t[:], start=True, stop=True)
                    msr_sb = med.tile([2 * DF, NV], f32r, tag="msr")
                    nc.scalar.copy(msr_sb[:], ps_msr[:])
                    # transpose Msr -> [128v, 4c, 64]
                    ps_mt = pmiscp.tile([128, VC, 2 * DF], f32r, tag="pmsr")
                    for c in range(VC):
                        nc.tensor.transpose(ps_mt[:, c, :],
                                            msr_sb[:, c * 128:(c + 1) * 128],
                                            ident_r[0:2 * DF, 0:2 * DF])
                    msrT = med.tile([128, VC, 2 * DF], f32r, tag="msrT")
                    nc.vector.tensor_copy(msrT[:], ps_mt[:])

                    oe_t = med.tile([DF, NE], f32r, tag="oe")
                    oeacc = small.tile([DF, ET], f32, tag="oeacc")
                    pagg = paggp.tile([DF, NV], f32, tag="pagg")

                    for et in range(ET):
                        es_t = big.tile([128, VC, ETW], f32r, tag="es")
                        er_t = big.tile([128, VC, ETW], f32r, tag="er")
                        src_s = E_s.ap()[g].rearrange("(c p) e -> p c e", p=128)
                        src_r = E_r.ap()[g].rearrange("(c p) e -> p c e", p=128)
                        nc.sync.dma_start(
                            es_t[:], src_s[:, :, et * ETW:(et + 1) * ETW])
                        nc.sync.dma_start(
                            er_t[:], src_r[:, :, et * ETW:(et + 1) * ETW])

                        # edge L1 (accumulated): W1_Ea@Ea + MsT@Es + MrT@Er
                        ph = php.tile([DF, ETW], f32, tag="ph")
                        nc.tensor.matmul(ph[:], W[:, FE_EA, :],
                                         ea_t[:, et * ETW:(et + 1) * ETW],
                                         start=True, stop=False)
                        for c in range(VC):
                            nc.tensor.matmul(ph[:], msrT[:, c, 0:DF],
                                             es_t[:, c, :],
                                             start=False, stop=False)
                        for c in range(VC):
                            nc.tensor.matmul(ph[:], msrT[:, c, DF:2 * DF],
                                             er_t[:, c, :],
                                             start=False, stop=(c == VC - 1))
                        h = small.tile([DF, ETW], f32r, tag="h")
                        nc.scalar.activation(h[:], ph[:], AF.Gelu,
                                             bias=b1e_all[:, g:g + 1])
                        for li, (wi, bi) in enumerate(((FE_W2, 1), (FE_W3, 2))):
                            ph2 = php.tile([DF, ETW], f32, tag="ph")
                            nc.tensor.matmul(ph2[:], W[:, wi, :], h[:],
                                             start=True, stop=True)
                            h = small.tile([DF, ETW], f32r, tag="h")
                            nc.scalar.activation(h[:], ph2[:], AF.Gelu,
                                                 bias=BIt[:, bi:bi + 1])
                        ph4 = php.tile([DF, ETW], f32, tag="ph")
                        nc.tensor.matmul(ph4[:], W[:, FE_W4, :], h[:],
                                         start=True, stop=True)
                        nc.scalar.activation(oe_t[:, et * ETW:(et + 1) * ETW],
                                             ph4[:], AF.Gelu,
                                             bias=BIt[:, 3:4],
                                             accum_out=oeacc[:, et:et + 1])

                        # transpose out_E tile -> [128e, 4c, 32]
                        ps_oet = poetp.tile([128, VC, DF], f32r, tag="poet")
                        for c in range(VC):
                            nc.tensor.transpose(
                                ps_oet[:, c, :],
                                oe_t[:, et * ETW + c * 128:et * ETW + (c + 1) * 128],
                                ident_r[0:DF, 0:DF])
                        oeT = small.tile([128, VC, DF], f32r, tag="oeT")
                        nc.vector.tensor_copy(oeT[:], ps_oet[:])

                        # agg += out_E_chunk @ E_r_chunk^T  (transpose E_r on PE)
                        for c in range(VC):
                            ps_ert = pertp.tile([128, VC, 128], f32r, tag="pert")
                            for vc in range(VC):
                                nc.tensor.transpose(
                                    ps_ert[:, vc, :],
                                    er_t[:, vc, c * 128:(c + 1) * 128],
                                    ident_r[:])
                            erT = small.tile([128, VC, 128], f32r, tag="erT")
                            nc.vector.tensor_copy(erT[:], ps_ert[:])
                            k = et * VC + c
                            nc.tensor.matmul(
                                pagg[:], oeT[:, c, :],
                                erT[:].rearrange("p c e -> p (c e)"),
                                start=(k == 0), stop=(k == ET * VC - 1),
                                skip_group_check=True)

                    nc.sync.dma_start(out_E.ap()[g], oe_t[:])
                    # out_E per-graph sum over edges
                    nc.vector.reduce_sum(oEsum[:, g:g + 1], oeacc[:],
                                         axis=mybir.AxisListType.X)

                    # node MLP: L1 = W1_Va@V_a + W1_agg@agg (+ folded u bias)
                    agg_sb = small.tile([DF, NV], f32r, tag="agg")
                    nc.scalar.copy(agg_sb[:], pagg[:])
                    phn = php.tile([DF, NV], f32, tag="ph")
                    nc.tensor.matmul(phn[:], W[:, FV_VA, :], va_t[:],
                                     start=True, stop=False)
                    nc.tensor.matmul(phn[:], W[:, FV_AGG, :], agg_sb[:],
                                     start=False, stop=True)
                    hn = small.tile([DF, NV], f32r, tag="h")
                    nc.scalar.activation(hn[:], phn[:], AF.Gelu,
                                         bias=b1v_all[:, g:g + 1])
                    for wi, bi in ((FV_W2, 5), (FV_W3, 6)):
                        phn2 = php.tile([DF, NV], f32, tag="ph")
                        nc.tensor.matmul(phn2[:], W[:, wi, :], hn[:],
                                         start=True, stop=True)
                        hn = small.tile([DF, NV], f32r, tag="h")
                        nc.scalar.activation(hn[:], phn2[:], AF.Gelu,
                                             bias=BIt[:, bi:bi + 1])
                    phn4 = php.tile([DF, NV], f32, tag="ph")
                    nc.tensor.matmul(phn4[:], W[:, FV_W4, :], hn[:],
                                     start=True, stop=True)
                    ov_t = med.tile([DF, NV], f32r, tag="ov")
                    nc.scalar.activation(ov_t[:], phn4[:], AF.Gelu,
                                         bias=BIt[:, 7:8],
                                         accum_out=oVsum[:, g:g + 1])
                    nc.sync.dma_start(out_V.ap()[g], ov_t[:])

                # global MLP over all graphs at once: P = [u; sumV; sumE]
                vs_r = work.tile([DF, GPC], f32r, tag="vs_r")
                es_r = work.tile([DF, GPC], f32r, tag="es_r")
                nc.vector.tensor_copy(vs_r[:], oVsum[:])
                nc.vector.tensor_copy(es_r[:], oEsum[:])
                phu = php.tile([DF, GPC], f32, tag="ph")
                nc.tensor.matmul(phu[:], W[:, FU_U, :], u_all[:],
                                 start=True, stop=False)
                nc.tensor.matmul(phu[:], W[:, FU_VS, :], vs_r[:],
                                 start=False, stop=False)
                nc.tensor.matmul(phu[:], W[:, FU_ES, :], es_r[:],
                                 start=False, stop=True)
                hu = work.tile([DF, GPC], f32r, tag="hu")
                nc.scalar.activation(hu[:], phu[:], AF.Gelu, bias=BIt[:, 8:9])
                for wi, bi in ((FU_W2, 9), (FU_W3, 10)):
                    phu2 = php.tile([DF, GPC], f32, tag="ph")
                    nc.tensor.matmul(phu2[:], W[:, wi, :], hu[:],
                                     start=True, stop=True)
                    hu = work.tile([DF, GPC], f32r, tag="hu")
                    nc.scalar.activation(hu[:], phu2[:], AF.Gelu,
                                         bias=BIt[:, bi:bi + 1])
                phu4 = php.tile([DF, GPC], f32, tag="ph")
                nc.tensor.matmul(phu4[:], W[:, FU_W4, :], hu[:],
                                 start=True, stop=True)
                ou_t = work.tile([DF, GPC], f32r, tag="ou")
                nc.scalar.activation(ou_t[:], phu4[:], AF.Gelu,
                                     bias=BIt[:, 11:12])
                nc.sync.dma_start(out_uT.ap(), ou_t[:])

    nc.compile()
    return nc


def _get_nc():
    if "nc" not in _cached:
        _cached["nc"] = _build_nc()
    return _cached["nc"]


def _pack_host(E_a, E_s, E_r, V_a, u, fe_Ws, fe_bs, fv_Ws, fv_bs, fu_Ws, fu_bs):
    """Build the 19-matrix stationary pack + 12 biases."""
    f = np.float32
    fe_Ws = [np.asarray(w, f) for w in fe_Ws]
    fv_Ws = [np.asarray(w, f) for w in fv_Ws]
    fu_Ws = [np.asarray(w, f) for w in fu_Ws]
    W1e, W1v, W1u = fe_Ws[0], fv_Ws[0], fu_Ws[0]
    mats = [
        W1e[:, 0:32].T, W1e[:, 32:64].T, W1e[:, 64:96].T, W1e[:, 96:128].T,
        fe_Ws[1].T, fe_Ws[2].T, fe_Ws[3].T,
        W1v[:, 0:32].T, W1v[:, 32:64].T, W1v[:, 64:96].T,
        fv_Ws[1].T, fv_Ws[2].T, fv_Ws[3].T,
        W1u[:, 0:32].T, W1u[:, 32:64].T, W1u[:, 64:96].T,
        fu_Ws[1].T, fu_Ws[2].T, fu_Ws[3].T,
    ]
    Wk = np.ascontiguousarray(np.stack(mats, axis=1), dtype=f)   # [32, 19, 32]
    bs = [np.asarray(b, f) for b in (*fe_bs, *fv_bs, *fu_bs)]
    BI = np.ascontiguousarray(np.stack(bs, axis=1), dtype=f)     # [32, 12]
    return Wk, BI


def kernel(E_a, E_s, E_r, V_a, u, fe_Ws, fe_bs, fv_Ws, fv_bs, fu_Ws, fu_bs,
           _want_trace=False):
    from concourse import bass_utils

    f = np.float32
    E_a = np.ascontiguousarray(np.asarray(E_a, f))
    E_s = np.ascontiguousarray(np.asarray(E_s, f))
    E_r = np.ascontiguousarray(np.asarray(E_r, f))
    V_a = np.ascontiguousarray(np.asarray(V_a, f))
    u = np.ascontiguousarray(np.asarray(u, f))
    Wk, BI = _pack_host(E_a, E_s, E_r, V_a, u, fe_Ws, fe_bs, fv_Ws, fv_bs,
                        fu_Ws, fu_bs)

    nc = _get_nc()
    in_maps = []
    for i in range(N_CORES):
        sl = slice(i * GPC, (i + 1) * GPC)
        in_maps.append({
            "E_s": E_s[sl], "E_r": E_r[sl], "E_a": E_a[sl], "V_a": V_a[sl],
            "uT": np.ascontiguousarray(u[sl].T), "Wk": Wk, "BI": BI,
        })
    res = bass_utils.run_bass_kernel_spmd(
        nc, in_maps, core_ids=list(range(N_CORES)), trace=_want_trace)

    out_E = np.concatenate([res.results[i]["out_E"] for i in range(N_CORES)], 0)
    out_V = np.concatenate([res.results[i]["out_V"] for i in range(N_CORES)], 0)
    out_u = np.concatenate([res.results[i]["out_uT"].T for i in range(N_CORES)],
                           0)
    out = (out_E.astype(f), out_V.astype(f), out_u.astype(f))
    if _want_trace:
        _cached["last_result"] = res
    return out
